# Initial kernel scaffold
#
"""Trainium2 Bass kernel for nn_Detail_loss (histogram_binning).

Data-parallel over B=32 samples -> 8 cores x 4 samples. Each core:
  1. 5x5 binary dilation of labels -> mask (PE banded matmuls vertical,
     row-cumsum difference trick horizontal).
  2. Masked 256-bin histogram of images*mask*255 (torch.histc semantics)
     via 16x16 hi/lo one-hot factorization: hist2d[h,l] = sum_p
     m_p*[hi_p==h]*[lo_p==l], computed as PE outer-product matmuls over
     bf16 one-hot planes.
  3. Two-threshold Otsu argmax over the 254x254 grid (first max,
     row-major), exact f32 divides.
  4. ci = where(im>=t2, 1, where(im>=t1, 0.5, 0)); per-sample
     sq = sum((ci - preds*mask)^2), sm = sum(mask).
Host: loss = mean over valid samples of sq/sm (np.float32 math).
"""

import numpy as np

import concourse.bass as bass
import concourse.mybir as mybir
from concourse import bacc, tile
from concourse.bass_utils import run_bass_kernel_spmd

F32 = mybir.dt.float32
BF16 = mybir.dt.bfloat16
I32 = mybir.dt.int32
OP = mybir.AluOpType
ACT = mybir.ActivationFunctionType
AX = mybir.AxisListType

B_PER_CORE = 4
H = 512
W = 512
NSLAB = 4          # 512 rows / 128 partitions
NBINS = 256
NT = 254
BIG = 1.0e8
EPS = 1e-8

# f32 constants replicated exactly from the reference arithmetic
C_BIN = np.float32(NBINS / 255.0)            # fl(256/255)
S1 = np.float32(255.0)


def build_nc():
    nc = bacc.Bacc("TRN2", target_bir_lowering=False)

    lab_d = nc.dram_tensor("labels", [B_PER_CORE * H, W], F32, kind="ExternalInput")
    img_d = nc.dram_tensor("images", [B_PER_CORE * H, W], F32, kind="ExternalInput")
    prd_d = nc.dram_tensor("preds", [B_PER_CORE * H, W], F32, kind="ExternalInput")
    # out[0, 4b+s]      = partial sq  (sample b, slab s, pre-summed over partitions)
    # out[0, 16+4b+s]   = partial sm
    out_d = nc.dram_tensor("stats", [1, 32], F32, kind="ExternalOutput")

    with tile.TileContext(nc) as tc:
        _emit(nc, tc, lab_d, img_d, prd_d, out_d)
    nc.compile()
    return nc


def _sample_view(dram, b):
    # rows [512b, 512b+512) viewed as SBUF-shaped [128, 4*512]
    return dram[512 * b:512 * (b + 1), :].rearrange("(s p) c -> p (s c)", p=128)


def _emit(nc, tc, lab_d, img_d, prd_d, out_d):
    import contextlib
    ctx = contextlib.ExitStack()
    with ctx:
        const = ctx.enter_context(tc.tile_pool(name="const", bufs=1))
        lab_pool = ctx.enter_context(tc.tile_pool(name="lab", bufs=2))
        labb_pool = ctx.enter_context(tc.tile_pool(name="labb", bufs=2))
        img_pool = ctx.enter_context(tc.tile_pool(name="img", bufs=2))
        prd_pool = ctx.enter_context(tc.tile_pool(name="prd", bufs=2))
        m_pool = ctx.enter_context(tc.tile_pool(name="mask", bufs=2))
        mb_pool = ctx.enter_context(tc.tile_pool(name="maskb", bufs=2))
        scr_pool = ctx.enter_context(tc.tile_pool(name="scr", bufs=2))
        plane_pool = ctx.enter_context(tc.tile_pool(name="planes", bufs=2))
        otsu_pool = ctx.enter_context(tc.tile_pool(name="otsu", bufs=2))
        stat_pool = ctx.enter_context(tc.tile_pool(name="stat", bufs=1))
        vpsum = ctx.enter_context(
            tc.tile_pool(name="vpsum", bufs=2, space=bass.MemorySpace.PSUM))
        hpsum = ctx.enter_context(
            tc.tile_pool(name="hpsum", bufs=2, space=bass.MemorySpace.PSUM))

        # ---------------- constants ----------------
        io_fp = const.tile([128, 128], I32, tag="io_fp")   # f - p
        nc.gpsimd.iota(io_fp[:], pattern=[[1, 128]], base=0, channel_multiplier=-1)
        io_pf = const.tile([128, 128], I32, tag="io_pf")   # p - f
        nc.gpsimd.iota(io_pf[:], pattern=[[-1, 128]], base=0, channel_multiplier=1)

        bv_band = const.tile([128, 128], BF16, tag="bv_band")
        nc.vector.tensor_scalar(bv_band[:], io_fp[:], 0, 2, OP.abs_max, OP.is_le)
        up_band = const.tile([128, 128], BF16, tag="up_band")
        nc.vector.tensor_scalar(up_band[:], io_pf[:], 126, None, OP.is_ge)
        dn_band = const.tile([128, 128], BF16, tag="dn_band")
        nc.vector.tensor_scalar(dn_band[:], io_fp[:], 126, None, OP.is_ge)

        io256 = const.tile([1, 256], F32, tag="io256")     # 0..255
        nc.gpsimd.iota(io256[:], pattern=[[1, 256]], base=0, channel_multiplier=0,
                       allow_small_or_imprecise_dtypes=True)
        # t2 index + BIG, replicated on 127 partitions
        iobig = const.tile([127, NT], F32, tag="iobig")
        nc.gpsimd.iota(iobig[:], pattern=[[1, NT]], base=0, channel_multiplier=0,
                       allow_small_or_imprecise_dtypes=True)
        nc.vector.tensor_scalar(iobig[:], iobig[:], BIG, None, OP.add)
        # flat-index base: 254*p + 127*254*h   (h = half index 0/1)
        fbase = const.tile([127, 2], F32, tag="fbase")
        nc.gpsimd.iota(fbase[:], pattern=[[127 * 254, 2]], base=0,
                       channel_multiplier=254, allow_small_or_imprecise_dtypes=True)

        # per-(sample,slab) stat columns
        sq_cols = stat_pool.tile([128, 16], F32, tag="sq_cols")
        sm_cols = stat_pool.tile([128, 16], F32, tag="sm_cols")

        for b in range(B_PER_CORE):
            # ---------------- load ----------------
            lab = lab_pool.tile([128, 4 * W], F32, tag="lab")
            nc.sync.dma_start(out=lab[:], in_=_sample_view(lab_d, b))
            img = img_pool.tile([128, 4 * W], F32, tag="img")
            nc.sync.dma_start(out=img[:], in_=_sample_view(img_d, b))

            labb = labb_pool.tile([128, 4 * W], BF16, tag="labb")
            for s in range(NSLAB):
                nc.scalar.activation(labb[:, 512 * s:512 * (s + 1)],
                                     lab[:, 512 * s:512 * (s + 1)], ACT.Copy)

            M = m_pool.tile([128, 4 * W], F32, tag="M")
            Mb = mb_pool.tile([128, 4 * W], BF16, tag="Mb")

            hist = hpsum.tile([16, 16], F32, tag="hist")

            for s in range(NSLAB):
                sl = slice(512 * s, 512 * (s + 1))
                # ------- vertical 5-conv (PE banded) -------
                yv = vpsum.tile([128, W], F32, tag="yv")
                mms = [(bv_band, s)]
                if s > 0:
                    mms.append((up_band, s - 1))
                if s < NSLAB - 1:
                    mms.append((dn_band, s + 1))
                for i, (band, src) in enumerate(mms):
                    nc.tensor.matmul(
                        yv[:], band[:], labb[:, 512 * src:512 * (src + 1)],
                        start=(i == 0), stop=(i == len(mms) - 1))

                # ------- horizontal via row-cumsum difference -------
                cp = scr_pool.tile([128, 520], F32, tag="cp")
                nc.vector.memset(cp[:, 0:3], 0.0)
                nc.vector.tensor_tensor_scan(
                    cp[:, 3:515], yv[:], yv[:], 0.0, OP.add, OP.bypass)
                nc.vector.tensor_copy(out=cp[:, 515:517], in_=cp[:, 514:515].rearrange("p (a c) -> p (a c)", a=1).broadcast(1, 2))
                # mask: C[c+2] > C[c-3]  (indices in cp: hi = c+5, lo = c)
                nc.vector.scalar_tensor_tensor(
                    M[:, sl], cp[:, 5:517], 0.0, cp[:, 0:512],
                    OP.add, OP.is_gt,
                    accum_out=sm_cols[:, 4 * b + s:4 * b + s + 1])
                nc.scalar.activation(Mb[:, sl], M[:, sl], ACT.Copy)

                # ------- bin index (exact reference arithmetic) -------
                # im0 = images*mask (in place into img tile)
                nc.vector.tensor_tensor(img[:, sl], img[:, sl], M[:, sl], OP.mult)
                w = scr_pool.tile([128, W], F32, tag="w")
                # w = (im0*255)*C_BIN, two roundings like the reference
                nc.vector.tensor_scalar(w[:], img[:, sl], S1, C_BIN, OP.mult, OP.mult)
                fr = scr_pool.tile([128, W], F32, tag="fr")
                nc.gpsimd.tensor_scalar(fr[:], w[:], 1.0, None, OP.mod)
                idx = scr_pool.tile([128, W], F32, tag="idx")
                nc.vector.scalar_tensor_tensor(idx[:], fr[:], -1.0, w[:], OP.mult, OP.add)
                nc.vector.tensor_scalar(idx[:], idx[:], 255.0, None, OP.min)
                q = scr_pool.tile([128, W], F32, tag="q")
                nc.scalar.activation(q[:], idx[:], ACT.Copy, scale=0.0625)
                f16 = scr_pool.tile([128, W], F32, tag="f16")
                nc.gpsimd.tensor_scalar(f16[:], q[:], 1.0, None, OP.mod)
                hi = scr_pool.tile([128, W], BF16, tag="hi")
                nc.vector.scalar_tensor_tensor(hi[:], f16[:], -1.0, q[:], OP.mult, OP.add)
                lo = scr_pool.tile([128, W], BF16, tag="lo")
                nc.vector.scalar_tensor_tensor(lo[:], hi[:], -16.0, idx[:], OP.mult, OP.add)

                # ------- one-hot planes (bf16) -------
                A = plane_pool.tile([128, 16 * W], BF16, tag="A")
                Bp = plane_pool.tile([128, 16 * W], BF16, tag="B")
                for j in range(16):
                    pl = slice(512 * j, 512 * (j + 1))
                    nc.vector.scalar_tensor_tensor(
                        A[:, pl], hi[:], float(j), Mb[:, sl], OP.is_equal, OP.mult)
                for j in range(16):
                    pl = slice(512 * j, 512 * (j + 1))
                    nc.gpsimd.tensor_scalar(
                        Bp[:, pl], lo[:], float(j), None, OP.is_equal)

                # ------- PE outer-product accumulation -------
                Ac = A[:].rearrange("p (j c) -> p c j", j=16)
                Bc = Bp[:].rearrange("p (j c) -> p c j", j=16)
                for c in range(W):
                    nc.tensor.matmul(
                        hist[:], Ac[:, c, :], Bc[:, c, :],
                        start=(s == 0 and c == 0),
                        stop=(s == NSLAB - 1 and c == W - 1))

            # ---------------- Otsu ----------------
            hrow = otsu_pool.tile([1, 256], F32, tag="hrow")
            nc.sync.dma_start(out=hrow[:], in_=hist[:].rearrange("p c -> () (p c)"))
            ntot = otsu_pool.tile([1, 1], F32, tag="ntot")
            nc.vector.tensor_reduce(ntot[:], hrow[:], AX.X, OP.add)
            hn = otsu_pool.tile([1, 256], F32, tag="hn")
            nc.vector.tensor_scalar(hn[:], hrow[:], ntot[:], None, OP.divide)
            ch = otsu_pool.tile([1, 256], F32, tag="ch")
            nc.vector.tensor_tensor_scan(ch[:], hn[:], hn[:], 0.0, OP.add, OP.bypass)
            hj = otsu_pool.tile([1, 256], F32, tag="hj")
            nc.vector.tensor_tensor(hj[:], hn[:], io256[:], OP.mult)
            cm = otsu_pool.tile([1, 256], F32, tag="cm")
            nc.vector.tensor_tensor_scan(cm[:], hj[:], hj[:], 0.0, OP.add, OP.bypass)

            # transposed columns a(t1), b(t1) per half + tm broadcast
            acol = otsu_pool.tile([127, 2], F32, tag="acol")
            bcol = otsu_pool.tile([127, 2], F32, tag="bcol")
            for h in range(2):
                rs = slice(127 * h, 127 * (h + 1))
                nc.sync.dma_start(out=acol[:, h:h + 1],
                                  in_=ch[0:1, rs].rearrange("a c -> c a"))
                nc.sync.dma_start(out=bcol[:, h:h + 1],
                                  in_=cm[0:1, rs].rearrange("a c -> c a"))
            tmcol = otsu_pool.tile([127, 1], F32, tag="tmcol")
            nc.gpsimd.partition_broadcast(tmcol[:], cm[0:1, 255:256], channels=127)
            ab = otsu_pool.tile([127, NT], F32, tag="ab")
            nc.gpsimd.partition_broadcast(ab[:], ch[0:1, 0:NT], channels=127)
            bb = otsu_pool.tile([127, NT], F32, tag="bb")
            nc.gpsimd.partition_broadcast(bb[:], cm[0:1, 0:NT], channels=127)

            colmax2 = otsu_pool.tile([127, 2], F32, tag="colmax2")
            t2min2 = otsu_pool.tile([127, 2], F32, tag="t2min2")
            for h in range(2):
                a_c = acol[:, h:h + 1]
                b_c = bcol[:, h:h + 1]
                g = otsu_pool.tile([127, NT], F32, tag="g_w1")
                w1 = otsu_pool.tile([127, NT], F32, tag="w1")
                nc.vector.tensor_scalar(w1[:], ab[:], a_c, None, OP.subtract)
                w1p = otsu_pool.tile([127, NT], F32, tag="w1p")
                nc.vector.tensor_scalar(w1p[:], w1[:], EPS, None, OP.add)
                vw = otsu_pool.tile([127, NT], F32, tag="vw")
                nc.vector.tensor_scalar(vw[:], w1[:], 0.0, None, OP.is_gt)
                m1m0 = otsu_pool.tile([127, NT], F32, tag="m1m0")
                nc.vector.tensor_scalar(m1m0[:], bb[:], b_c, None, OP.subtract)
                mean1 = otsu_pool.tile([127, NT], F32, tag="mean1")
                nc.vector.tensor_tensor(mean1[:], m1m0[:], w1p[:], OP.divide)
                d1 = otsu_pool.tile([127, NT], F32, tag="d1")
                nc.vector.tensor_scalar(d1[:], mean1[:], tmcol[:], None, OP.subtract)
                nc.vector.tensor_tensor(d1[:], d1[:], d1[:], OP.mult)
                bv1 = otsu_pool.tile([127, NT], F32, tag="bv1")
                nc.vector.tensor_tensor(bv1[:], d1[:], w1[:], OP.mult)

                w2 = otsu_pool.tile([127, NT], F32, tag="w2")
                nc.vector.tensor_scalar(w2[:], ab[:], -1.0, 1.0, OP.mult, OP.add)
                w2p = otsu_pool.tile([127, NT], F32, tag="w2p")
                nc.vector.tensor_scalar(w2p[:], w2[:], EPS, None, OP.add)
                vw2 = otsu_pool.tile([127, NT], F32, tag="vw2")
                nc.vector.scalar_tensor_tensor(vw2[:], w2[:], 0.0, vw[:], OP.is_gt, OP.mult)
                tmm1 = otsu_pool.tile([127, NT], F32, tag="tmm1")
                nc.vector.tensor_scalar(tmm1[:], bb[:], -1.0, tmcol[:], OP.mult, OP.add)
                mean2 = otsu_pool.tile([127, NT], F32, tag="mean2")
                nc.vector.tensor_tensor(mean2[:], tmm1[:], w2p[:], OP.divide)
                d2 = otsu_pool.tile([127, NT], F32, tag="d2")
                nc.vector.tensor_scalar(d2[:], mean2[:], tmcol[:], None, OP.subtract)
                nc.vector.tensor_tensor(d2[:], d2[:], d2[:], OP.mult)
                bv2 = otsu_pool.tile([127, NT], F32, tag="bv2")
                nc.vector.tensor_tensor(bv2[:], d2[:], w2[:], OP.mult)

                # column (t1-only) terms: w0 = a, mass0 = b
                w0p = otsu_pool.tile([127, 1], F32, tag="w0p")
                nc.vector.tensor_scalar(w0p[:], a_c, EPS, None, OP.add)
                mean0 = otsu_pool.tile([127, 1], F32, tag="mean0")
                nc.vector.tensor_tensor(mean0[:], b_c, w0p[:], OP.divide)
                d0 = otsu_pool.tile([127, 1], F32, tag="d0")
                nc.vector.tensor_scalar(d0[:], mean0[:], tmcol[:], None, OP.subtract)
                nc.vector.tensor_tensor(d0[:], d0[:], d0[:], OP.mult)
                bv0 = otsu_pool.tile([127, 1], F32, tag="bv0")
                nc.vector.tensor_tensor(bv0[:], d0[:], a_c, OP.mult)
                vw0 = otsu_pool.tile([127, 1], F32, tag="vw0")
                nc.vector.tensor_scalar(vw0[:], a_c, 0.0, None, OP.is_gt)

                bv = otsu_pool.tile([127, NT], F32, tag="bv")
                nc.vector.tensor_tensor(bv[:], bv1[:], bv2[:], OP.add)
                nc.vector.tensor_scalar(bv[:], bv[:], bv0[:], None, OP.add)
                nc.vector.tensor_tensor(bv[:], bv[:], vw2[:], OP.mult)
                nc.vector.tensor_scalar(bv[:], bv[:], vw0[:], None, OP.mult)

                cmx = colmax2[:, h:h + 1]
                nc.vector.tensor_reduce(cmx, bv[:], AX.X, OP.max)
                eq = otsu_pool.tile([127, NT], F32, tag="eq")
                nc.vector.tensor_scalar(eq[:], bv[:], cmx, None, OP.is_equal)
                cand = otsu_pool.tile([127, NT], F32, tag="cand")
                nc.vector.scalar_tensor_tensor(
                    cand[:], eq[:], -BIG, iobig[:], OP.mult, OP.add)
                nc.vector.tensor_reduce(t2min2[:, h:h + 1], cand[:], AX.X, OP.min)
                del g

            # global first-max across [127, 2]
            gmax = otsu_pool.tile([127, 1], F32, tag="gmax")
            nc.vector.tensor_reduce(gmax[:], colmax2[:], AX.X, OP.max)
            nc.gpsimd.partition_all_reduce(gmax[:], gmax[:], channels=127,
                                           reduce_op=bass.bass_isa.ReduceOp.max)
            flat = otsu_pool.tile([127, 2], F32, tag="flat")
            nc.vector.tensor_tensor(flat[:], t2min2[:], fbase[:], OP.add)
            nfb = otsu_pool.tile([127, 2], F32, tag="nfb")
            nc.vector.tensor_scalar(nfb[:], flat[:], -1.0, -BIG, OP.mult, OP.add)
            eqg = otsu_pool.tile([127, 2], F32, tag="eqg")
            nc.vector.tensor_scalar(eqg[:], colmax2[:], gmax[:], None, OP.is_equal)
            nf = otsu_pool.tile([127, 2], F32, tag="nf")
            nc.vector.scalar_tensor_tensor(nf[:], eqg[:], BIG, nfb[:], OP.mult, OP.add)
            nfm = otsu_pool.tile([127, 1], F32, tag="nfm")
            nc.vector.tensor_reduce(nfm[:], nf[:], AX.X, OP.max)
            nc.gpsimd.partition_all_reduce(nfm[:], nfm[:], channels=127,
                                           reduce_op=bass.bass_isa.ReduceOp.max)

            fl1 = otsu_pool.tile([1, 1], F32, tag="fl1")
            nc.vector.tensor_scalar(fl1[:], nfm[0:1, 0:1], -1.0, None, OP.mult)
            # t1 = floor((flat+0.5)/254); t2 = flat - 254*t1
            qt = otsu_pool.tile([1, 1], F32, tag="qt")
            nc.vector.tensor_scalar(qt[:], fl1[:], 0.5, 254.0, OP.add, OP.divide)
            qf = otsu_pool.tile([1, 1], F32, tag="qf")
            nc.vector.tensor_scalar(qf[:], qt[:], 1.0, None, OP.mod)
            t1i = otsu_pool.tile([1, 1], F32, tag="t1i")
            nc.vector.scalar_tensor_tensor(t1i[:], qf[:], -1.0, qt[:], OP.mult, OP.add)
            t2i = otsu_pool.tile([1, 1], F32, tag="t2i")
            nc.vector.scalar_tensor_tensor(t2i[:], t1i[:], -254.0, fl1[:], OP.mult, OP.add)
            # thresholds: (t + 1)/255 with true division
            T1 = otsu_pool.tile([1, 1], F32, tag="T1")
            nc.vector.tensor_scalar(T1[:], t1i[:], 1.0, 255.0, OP.add, OP.divide)
            T2 = otsu_pool.tile([1, 1], F32, tag="T2")
            nc.vector.tensor_scalar(T2[:], t2i[:], 1.0, 255.0, OP.add, OP.divide)
            T1c = otsu_pool.tile([128, 1], F32, tag="T1c")
            nc.gpsimd.partition_broadcast(T1c[:], T1[:], channels=128)
            T2c = otsu_pool.tile([128, 1], F32, tag="T2c")
            nc.gpsimd.partition_broadcast(T2c[:], T2[:], channels=128)

            # ---------------- MSE ----------------
            prd = prd_pool.tile([128, 4 * W], F32, tag="prd")
            nc.sync.dma_start(out=prd[:], in_=_sample_view(prd_d, b))
            for s in range(NSLAB):
                sl = slice(512 * s, 512 * (s + 1))
                ge1 = scr_pool.tile([128, W], F32, tag="ge1")
                nc.gpsimd.tensor_scalar(ge1[:], img[:, sl], T1c[:], None, OP.is_ge)
                ge2 = scr_pool.tile([128, W], F32, tag="ge2")
                nc.vector.tensor_scalar(ge2[:], img[:, sl], T2c[:], None, OP.is_ge)
                ci = scr_pool.tile([128, W], F32, tag="ci")
                nc.vector.scalar_tensor_tensor(ci[:], ge1[:], 0.5, ge2[:], OP.mult, OP.max)
                pm = scr_pool.tile([128, W], F32, tag="pm")
                nc.vector.tensor_tensor(pm[:], prd[:, sl], M[:, sl], OP.mult)
                d = scr_pool.tile([128, W], F32, tag="d")
                nc.vector.tensor_tensor(d[:], ci[:], pm[:], OP.subtract)
                dsq = scr_pool.tile([128, W], F32, tag="dsq")
                nc.vector.tensor_tensor_reduce(
                    dsq[:], d[:], d[:], 1.0, 0.0, OP.mult, OP.add,
                    accum_out=sq_cols[:, 4 * b + s:4 * b + s + 1])

        # ---------------- ship stats ----------------
        allc = stat_pool.tile([128, 32], F32, tag="allc")
        nc.vector.tensor_copy(out=allc[:, 0:16], in_=sq_cols[:])
        nc.vector.tensor_copy(out=allc[:, 16:32], in_=sm_cols[:])
        red = stat_pool.tile([1, 32], F32, tag="red")
        nc.gpsimd.tensor_reduce(red[:], allc[:], AX.C, OP.add)
        nc.sync.dma_start(out=out_d[:], in_=red[:])


_NC_CACHE = None


def _get_nc():
    global _NC_CACHE
    if _NC_CACHE is None:
        _NC_CACHE = build_nc()
    return _NC_CACHE


def kernel(preds, labels, images):
    preds = np.asarray(preds)
    labels = np.asarray(labels)
    images = np.asarray(images)
    B = preds.shape[0]
    assert B == 32 and preds.shape == (32, 1, 512, 512)
    nc = _get_nc()

    in_maps = []
    for c in range(8):
        sl = slice(B_PER_CORE * c, B_PER_CORE * (c + 1))
        in_maps.append({
            "labels": labels[sl, 0].reshape(B_PER_CORE * H, W),
            "images": images[sl, 0].reshape(B_PER_CORE * H, W),
            "preds": preds[sl, 0].reshape(B_PER_CORE * H, W),
        })
    res = run_bass_kernel_spmd(nc, in_maps, list(range(8)))

    # host: finish per-sample loss + mean over valid (np.float32 math)
    sq = np.zeros(32, np.float32)
    sm = np.zeros(32, np.float32)
    for c in range(8):
        st = res.results[c]["stats"][0]
        for b in range(B_PER_CORE):
            sq[B_PER_CORE * c + b] = np.sum(st[4 * b:4 * b + 4], dtype=np.float32)
            sm[B_PER_CORE * c + b] = np.sum(st[16 + 4 * b:16 + 4 * b + 4], dtype=np.float32)
    smp = sm + np.float32(EPS)
    valid = smp > np.float32(1e-8)
    loss_per = sq / smp
    cnt = np.float32(valid.sum())
    if cnt > 0:
        total = np.sum(np.where(valid, loss_per, np.float32(0.0)), dtype=np.float32)
        out = total / np.maximum(cnt, np.float32(1.0))
    else:
        out = np.float32(0.0)
    return np.float32(out)


# revision 9
# speedup vs baseline: 1.0090x; 1.0090x over previous
"""Trainium2 Bass kernel for nn_Detail_loss (histogram_binning).

Data-parallel over B=32 samples -> 8 cores x 4 samples. Each core:
  1. 5x5 binary dilation of labels -> mask (PE banded matmuls vertical,
     row-cumsum difference trick horizontal).
  2. Masked 256-bin histogram of images*mask*255 (torch.histc semantics)
     via 16x16 hi/lo one-hot factorization: hist2d[h,l] = sum_p
     m_p*[hi_p==h]*[lo_p==l], computed as PE outer-product matmuls over
     bf16 one-hot planes.
  3. Two-threshold Otsu argmax over the 254x254 grid (first max,
     row-major), exact f32 divides.
  4. ci = where(im>=t2, 1, where(im>=t1, 0.5, 0)); per-sample
     sq = sum((ci - preds*mask)^2), sm = sum(mask).
Host: loss = mean over valid samples of sq/sm (np.float32 math).
"""

import numpy as np

import concourse.bass as bass
import concourse.mybir as mybir
from concourse import bacc, bass_isa, tile
from concourse.bass_utils import run_bass_kernel_spmd

F32 = mybir.dt.float32
BF16 = mybir.dt.bfloat16
I32 = mybir.dt.int32
OP = mybir.AluOpType
ACT = mybir.ActivationFunctionType
AX = mybir.AxisListType

B_PER_CORE = 4
H = 512
W = 512
NSLAB = 4          # 512 rows / 128 partitions
NBINS = 256
NT = 254
BIG = 4194304.0  # 2^22: BIG+flat stays integer-exact in f32
EPS = 1e-8

# f32 constants replicated exactly from the reference arithmetic
C_BIN = float(np.float32(NBINS / 255.0))     # fl(256/255), exact in f64
S1 = 255.0


def build_nc():
    nc = bacc.Bacc("TRN2", target_bir_lowering=False)

    lab_d = nc.dram_tensor("labels", [B_PER_CORE * H, W], F32, kind="ExternalInput")
    img_d = nc.dram_tensor("images", [B_PER_CORE * H, W], F32, kind="ExternalInput")
    prd_d = nc.dram_tensor("preds", [B_PER_CORE * H, W], F32, kind="ExternalInput")
    # out[0, 4b+s]      = partial sq  (sample b, slab s, pre-summed over partitions)
    # out[0, 16+4b+s]   = partial sm
    out_d = nc.dram_tensor("stats", [1, 32], F32, kind="ExternalOutput")
    dbg_d = nc.dram_tensor("dbg", [1, 16], F32, kind="ExternalOutput")

    with tile.TileContext(nc) as tc:
        _emit(nc, tc, lab_d, img_d, prd_d, out_d, dbg_d)
    nc.compile()
    return nc


def _sample_view(dram, b):
    # rows [512b, 512b+512) viewed as SBUF-shaped [128, 4*512]
    return dram[512 * b:512 * (b + 1), :].rearrange("(s p) c -> p s c", p=128)


def _emit(nc, tc, lab_d, img_d, prd_d, out_d, dbg_d):
    import contextlib
    ctx = contextlib.ExitStack()
    with ctx:
        const = ctx.enter_context(tc.tile_pool(name="const", bufs=1))
        lab_pool = ctx.enter_context(tc.tile_pool(name="lab", bufs=2))
        labb_pool = ctx.enter_context(tc.tile_pool(name="labb", bufs=2))
        img_pool = ctx.enter_context(tc.tile_pool(name="img", bufs=2))
        prd_pool = ctx.enter_context(tc.tile_pool(name="prd", bufs=2))
        m_pool = ctx.enter_context(tc.tile_pool(name="mask", bufs=2))
        mb_pool = ctx.enter_context(tc.tile_pool(name="maskb", bufs=2))
        scr_pool = ctx.enter_context(tc.tile_pool(name="scr", bufs=2))
        plane_pool = ctx.enter_context(tc.tile_pool(name="planes", bufs=2))
        otsu_pool = ctx.enter_context(tc.tile_pool(name="otsu", bufs=1))
        stat_pool = ctx.enter_context(tc.tile_pool(name="stat", bufs=1))
        vpsum = ctx.enter_context(
            tc.tile_pool(name="vpsum", bufs=2, space=bass.MemorySpace.PSUM))
        hpsum = ctx.enter_context(
            tc.tile_pool(name="hpsum", bufs=2, space=bass.MemorySpace.PSUM))

        # ---------------- constants ----------------
        io_fp = const.tile([128, 128], I32, tag="io_fp")   # f - p
        nc.gpsimd.iota(io_fp[:], pattern=[[1, 128]], base=0, channel_multiplier=-1)
        io_pf = const.tile([128, 128], I32, tag="io_pf")   # p - f
        nc.gpsimd.iota(io_pf[:], pattern=[[-1, 128]], base=0, channel_multiplier=1)

        bv_band = const.tile([128, 128], BF16, tag="bv_band")
        btmp = const.tile([128, 128], F32, tag="btmp")
        nc.vector.tensor_scalar(btmp[:], io_fp[:], -2, None, OP.is_ge)
        nc.vector.scalar_tensor_tensor(bv_band[:], io_fp[:], 2, btmp[:], OP.is_le, OP.mult)
        up_band = const.tile([128, 128], BF16, tag="up_band")
        nc.vector.tensor_scalar(up_band[:], io_pf[:], 126, None, OP.is_ge)
        dn_band = const.tile([128, 128], BF16, tag="dn_band")
        nc.vector.tensor_scalar(dn_band[:], io_fp[:], 126, None, OP.is_ge)

        io256 = const.tile([1, 256], F32, tag="io256")     # 0..255
        nc.gpsimd.iota(io256[:], pattern=[[1, 256]], base=0, channel_multiplier=0,
                       allow_small_or_imprecise_dtypes=True)
        # t2 index + BIG, replicated on 127 partitions
        iobig = const.tile([127, NT], F32, tag="iobig")
        nc.gpsimd.iota(iobig[:], pattern=[[1, NT]], base=0, channel_multiplier=0,
                       allow_small_or_imprecise_dtypes=True)
        nc.vector.tensor_scalar(iobig[:], iobig[:], BIG, None, OP.add)
        # flat-index base: 254*p + 127*254*h   (h = half index 0/1)
        fbase = const.tile([127, 2], F32, tag="fbase")
        nc.gpsimd.iota(fbase[:], pattern=[[127 * 254, 2]], base=0,
                       channel_multiplier=254, allow_small_or_imprecise_dtypes=True)

        # per-(sample,slab) stat columns
        sq_cols = stat_pool.tile([128, 16], F32, tag="sq_cols")
        sm_cols = stat_pool.tile([128, 16], F32, tag="sm_cols")
        dbg_row = stat_pool.tile([1, 16], F32, tag="dbg_row")

        for b in range(B_PER_CORE):
            # ---------------- load ----------------
            lab = lab_pool.tile([128, 4 * W], F32, tag="lab")
            nc.sync.dma_start(out=lab[:].rearrange("p (s c) -> p s c", s=4),
                              in_=_sample_view(lab_d, b))
            img = img_pool.tile([128, 4 * W], F32, tag="img")
            nc.sync.dma_start(out=img[:].rearrange("p (s c) -> p s c", s=4),
                              in_=_sample_view(img_d, b))

            labb = labb_pool.tile([128, 4 * W], BF16, tag="labb")
            for s in range(NSLAB):
                nc.scalar.activation(labb[:, 512 * s:512 * (s + 1)],
                                     lab[:, 512 * s:512 * (s + 1)], ACT.Copy)

            M = m_pool.tile([128, 4 * W], F32, tag="M")

            hist = hpsum.tile([16, 16], F32, tag="hist")

            for s in range(NSLAB):
                sl = slice(512 * s, 512 * (s + 1))
                # ------- vertical 5-conv (PE banded) -------
                yv = vpsum.tile([128, W], F32, tag="yv")
                mms = [(bv_band, s)]
                if s > 0:
                    mms.append((up_band, s - 1))
                if s < NSLAB - 1:
                    mms.append((dn_band, s + 1))
                for i, (band, src) in enumerate(mms):
                    nc.tensor.matmul(
                        yv[:], band[:], labb[:, 512 * src:512 * (src + 1)],
                        start=(i == 0), stop=(i == len(mms) - 1))

                # ------- horizontal via row-cumsum difference -------
                cp = scr_pool.tile([128, 520], F32, tag="cp")
                nc.vector.memset(cp[:, 0:3], 0.0)
                nc.vector.tensor_tensor_scan(
                    cp[:, 3:515], yv[:], lab[:, sl], 0.0, OP.add, OP.bypass)
                nc.vector.tensor_copy(out=cp[:, 515:516], in_=cp[:, 514:515])
                nc.vector.tensor_copy(out=cp[:, 516:517], in_=cp[:, 514:515])
                # mask: C[c+2] > C[c-3]  (indices in cp: hi = c+5, lo = c)
                nc.vector.scalar_tensor_tensor(
                    M[:, sl], cp[:, 5:517], 0.0, cp[:, 0:512],
                    OP.add, OP.is_gt,
                    accum_out=sm_cols[:, 4 * b + s:4 * b + s + 1])
                Mb = mb_pool.tile([128, W], BF16, tag="Mb")
                nc.scalar.activation(Mb[:], M[:, sl], ACT.Copy)

                # ------- bin index (exact reference arithmetic) -------
                # im0 = images*mask (in place into img tile)
                nc.vector.tensor_tensor(img[:, sl], img[:, sl], M[:, sl], OP.mult)
                w = scr_pool.tile([128, W], F32, tag="t0")
                # w = (im0*255)*C_BIN, two roundings like the reference
                nc.vector.tensor_scalar(w[:], img[:, sl], S1, C_BIN, OP.mult, OP.mult)
                idx = scr_pool.tile([128, W], F32, tag="t1")
                nc.gpsimd.tensor_scalar(idx[:], w[:], 1.0, None, OP.mod)
                nc.vector.scalar_tensor_tensor(idx[:], idx[:], -1.0, w[:], OP.mult, OP.add)
                nc.vector.tensor_scalar(idx[:], idx[:], 255.0, None, OP.min)
                q = scr_pool.tile([128, W], F32, tag="t2")
                nc.scalar.activation(q[:], idx[:], ACT.Copy, scale=0.0625)
                f16 = scr_pool.tile([128, W], F32, tag="t3")
                nc.gpsimd.tensor_scalar(f16[:], q[:], 1.0, None, OP.mod)
                hi = scr_pool.tile([128, W], BF16, tag="hi")
                nc.vector.scalar_tensor_tensor(hi[:], f16[:], -1.0, q[:], OP.mult, OP.add)
                lo = scr_pool.tile([128, W], BF16, tag="lo")
                nc.vector.scalar_tensor_tensor(lo[:], hi[:], -16.0, idx[:], OP.mult, OP.add)

                # ------- one-hot planes (bf16) -------
                A = plane_pool.tile([128, 16 * W], BF16, tag="A")
                Bp = plane_pool.tile([128, 16 * W], BF16, tag="B")
                for j in range(16):
                    pl = slice(512 * j, 512 * (j + 1))
                    nc.vector.scalar_tensor_tensor(
                        A[:, pl], hi[:], float(j), Mb[:], OP.is_equal, OP.mult)
                for j in range(16):
                    pl = slice(512 * j, 512 * (j + 1))
                    nc.gpsimd.tensor_scalar(
                        Bp[:, pl], lo[:], float(j), None, OP.is_equal)

                # ------- PE outer-product accumulation -------
                Ac = A[:].rearrange("p (j c) -> p c j", j=16)
                Bc = Bp[:].rearrange("p (j c) -> p c j", j=16)
                for c in range(W):
                    nc.tensor.matmul(
                        hist[:], Ac[:, c, :], Bc[:, c, :],
                        start=(s == 0 and c == 0),
                        stop=(s == NSLAB - 1 and c == W - 1))

            # ---------------- Otsu ----------------
            hist_s = otsu_pool.tile([16, 16], F32, tag="hist_s")
            nc.vector.tensor_copy(out=hist_s[:], in_=hist[:])
            hrow = otsu_pool.tile([1, 256], F32, tag="hrow")
            nc.sync.dma_start(out=hrow[:], in_=hist_s[:])
            ntot = otsu_pool.tile([1, 1], F32, tag="ntot")
            nc.vector.tensor_reduce(ntot[:], hrow[:], AX.X, OP.add)
            hn = otsu_pool.tile([1, 256], F32, tag="hn")
            nc.vector.tensor_scalar(hn[:], hrow[:], ntot[:], None, OP.divide)
            ch = otsu_pool.tile([1, 256], F32, tag="ch")
            nc.vector.tensor_tensor_scan(ch[:], hn[:], hn[:], 0.0, OP.add, OP.bypass)
            hj = otsu_pool.tile([1, 256], F32, tag="hj")
            nc.vector.tensor_tensor(hj[:], hn[:], io256[:], OP.mult)
            cm = otsu_pool.tile([1, 256], F32, tag="cm")
            nc.vector.tensor_tensor_scan(cm[:], hj[:], hj[:], 0.0, OP.add, OP.bypass)

            # transposed columns a(t1), b(t1) per half + tm broadcast
            acol = otsu_pool.tile([127, 2], F32, tag="acol")
            bcol = otsu_pool.tile([127, 2], F32, tag="bcol")
            for h in range(2):
                rs = slice(127 * h, 127 * (h + 1))
                nc.sync.dma_start(out=acol[:, h:h + 1], in_=ch[0:1, rs])
                nc.sync.dma_start(out=bcol[:, h:h + 1], in_=cm[0:1, rs])
            tmcol = otsu_pool.tile([127, 1], F32, tag="tmcol")
            nc.gpsimd.partition_broadcast(tmcol[:], cm[0:1, 255:256], channels=127)
            ab = otsu_pool.tile([127, NT], F32, tag="ab")
            nc.gpsimd.partition_broadcast(ab[:], ch[0:1, 0:NT], channels=127)
            bb = otsu_pool.tile([127, NT], F32, tag="bb")
            nc.gpsimd.partition_broadcast(bb[:], cm[0:1, 0:NT], channels=127)

            colmax2 = otsu_pool.tile([127, 2], F32, tag="colmax2")
            t2min2 = otsu_pool.tile([127, 2], F32, tag="t2min2")
            for h in range(2):
                a_c = acol[:, h:h + 1]
                b_c = bcol[:, h:h + 1]
                w1 = otsu_pool.tile([127, NT], F32, tag="w1")
                nc.vector.tensor_scalar(w1[:], ab[:], a_c, None, OP.subtract)
                w1p = otsu_pool.tile([127, NT], F32, tag="w1p")
                nc.vector.tensor_scalar(w1p[:], w1[:], EPS, None, OP.add)
                vw = otsu_pool.tile([127, NT], F32, tag="vw")
                nc.vector.tensor_scalar(vw[:], w1[:], 0.0, None, OP.is_gt)
                m1m0 = otsu_pool.tile([127, NT], F32, tag="m1m0")
                nc.vector.tensor_scalar(m1m0[:], bb[:], b_c, None, OP.subtract)
                nc.vector.tensor_tensor(m1m0[:], m1m0[:], w1p[:], OP.divide)
                d1 = m1m0
                nc.vector.tensor_scalar(d1[:], d1[:], tmcol[:], None, OP.subtract)
                nc.vector.tensor_tensor(d1[:], d1[:], d1[:], OP.mult)
                bv1 = d1
                nc.vector.tensor_tensor(bv1[:], d1[:], w1[:], OP.mult)

                w2 = otsu_pool.tile([127, NT], F32, tag="w2")
                nc.vector.tensor_scalar(w2[:], ab[:], -1.0, 1.0, OP.mult, OP.add)
                w2p = otsu_pool.tile([127, NT], F32, tag="w2p")
                nc.vector.tensor_scalar(w2p[:], w2[:], EPS, None, OP.add)
                vw2 = otsu_pool.tile([127, NT], F32, tag="vw2")
                nc.vector.scalar_tensor_tensor(vw2[:], w2[:], 0.0, vw[:], OP.is_gt, OP.mult)
                tmm1 = otsu_pool.tile([127, NT], F32, tag="tmm1")
                nc.vector.tensor_scalar(tmm1[:], bb[:], -1.0, tmcol[:], OP.mult, OP.add)
                nc.vector.tensor_tensor(tmm1[:], tmm1[:], w2p[:], OP.divide)
                d2 = tmm1
                nc.vector.tensor_scalar(d2[:], d2[:], tmcol[:], None, OP.subtract)
                nc.vector.tensor_tensor(d2[:], d2[:], d2[:], OP.mult)
                bv2 = d2
                nc.vector.tensor_tensor(bv2[:], d2[:], w2[:], OP.mult)

                # column (t1-only) terms: w0 = a, mass0 = b
                w0p = otsu_pool.tile([127, 1], F32, tag="w0p")
                nc.vector.tensor_scalar(w0p[:], a_c, EPS, None, OP.add)
                d0 = otsu_pool.tile([127, 1], F32, tag="d0")
                nc.vector.tensor_tensor(d0[:], b_c, w0p[:], OP.divide)
                nc.vector.tensor_scalar(d0[:], d0[:], tmcol[:], None, OP.subtract)
                nc.vector.tensor_tensor(d0[:], d0[:], d0[:], OP.mult)
                bv0 = d0
                nc.vector.tensor_tensor(bv0[:], d0[:], a_c, OP.mult)
                vw0 = otsu_pool.tile([127, 1], F32, tag="vw0")
                nc.vector.tensor_scalar(vw0[:], a_c, 0.0, None, OP.is_gt)

                bv = otsu_pool.tile([127, NT], F32, tag="bv")
                nc.vector.tensor_scalar(bv[:], bv1[:], bv0[:], None, OP.add)
                nc.vector.tensor_tensor(bv[:], bv[:], bv2[:], OP.add)
                nc.vector.tensor_tensor(bv[:], bv[:], vw2[:], OP.mult)
                nc.vector.tensor_scalar(bv[:], bv[:], vw0[:], None, OP.mult)

                cmx = colmax2[:, h:h + 1]
                nc.vector.tensor_reduce(cmx, bv[:], AX.X, OP.max)
                eq = otsu_pool.tile([127, NT], F32, tag="eq")
                nc.vector.tensor_scalar(eq[:], bv[:], cmx, None, OP.is_equal)
                nc.vector.scalar_tensor_tensor(
                    eq[:], eq[:], -BIG, iobig[:], OP.mult, OP.add)
                nc.vector.tensor_reduce(t2min2[:, h:h + 1], eq[:], AX.X, OP.min)

            # global first-max across [127, 2]
            gmax = otsu_pool.tile([127, 1], F32, tag="gmax")
            nc.vector.tensor_reduce(gmax[:], colmax2[:], AX.X, OP.max)
            nc.gpsimd.partition_all_reduce(gmax[:], gmax[:], channels=127,
                                           reduce_op=bass_isa.ReduceOp.max)
            flat = otsu_pool.tile([127, 2], F32, tag="flat")
            nc.vector.tensor_tensor(flat[:], t2min2[:], fbase[:], OP.add)
            nfb = otsu_pool.tile([127, 2], F32, tag="nfb")
            nc.vector.tensor_scalar(nfb[:], flat[:], -1.0, -BIG, OP.mult, OP.add)
            eqg = otsu_pool.tile([127, 2], F32, tag="eqg")
            nc.vector.tensor_scalar(eqg[:], colmax2[:], gmax[:], None, OP.is_equal)
            nf = otsu_pool.tile([127, 2], F32, tag="nf")
            nc.vector.scalar_tensor_tensor(nf[:], eqg[:], BIG, nfb[:], OP.mult, OP.add)
            nfm = otsu_pool.tile([127, 1], F32, tag="nfm")
            nc.vector.tensor_reduce(nfm[:], nf[:], AX.X, OP.max)
            nc.gpsimd.partition_all_reduce(nfm[:], nfm[:], channels=127,
                                           reduce_op=bass_isa.ReduceOp.max)

            fl1 = otsu_pool.tile([1, 1], F32, tag="fl1")
            nc.vector.tensor_scalar(fl1[:], nfm[0:1, 0:1], -1.0, None, OP.mult)
            # t1 = floor((flat+0.5)/254); t2 = flat - 254*t1
            qt = otsu_pool.tile([1, 1], F32, tag="qt")
            nc.vector.tensor_scalar(qt[:], fl1[:], 0.5, 254.0, OP.add, OP.divide)
            qf = otsu_pool.tile([1, 1], F32, tag="qf")
            nc.vector.tensor_scalar(qf[:], qt[:], 1.0, None, OP.mod)
            t1i = otsu_pool.tile([1, 1], F32, tag="t1i")
            nc.vector.scalar_tensor_tensor(t1i[:], qf[:], -1.0, qt[:], OP.mult, OP.add)
            t2i = otsu_pool.tile([1, 1], F32, tag="t2i")
            nc.vector.scalar_tensor_tensor(t2i[:], t1i[:], -254.0, fl1[:], OP.mult, OP.add)
            # thresholds: (t + 1)/255 with true division
            T1 = otsu_pool.tile([1, 1], F32, tag="T1")
            nc.vector.tensor_scalar(T1[:], t1i[:], 1.0, 255.0, OP.add, OP.divide)
            T2 = otsu_pool.tile([1, 1], F32, tag="T2")
            nc.vector.tensor_scalar(T2[:], t2i[:], 1.0, 255.0, OP.add, OP.divide)
            nc.vector.tensor_copy(out=dbg_row[:, 4 * b:4 * b + 1], in_=fl1[:])
            nc.vector.tensor_copy(out=dbg_row[:, 4 * b + 1:4 * b + 2], in_=ntot[:])
            nc.vector.tensor_copy(out=dbg_row[:, 4 * b + 2:4 * b + 3], in_=T1[:])
            nc.vector.tensor_copy(out=dbg_row[:, 4 * b + 3:4 * b + 4], in_=T2[:])
            T1c = otsu_pool.tile([128, 1], F32, tag="T1c")
            nc.gpsimd.partition_broadcast(T1c[:], T1[:], channels=128)
            T2c = otsu_pool.tile([128, 1], F32, tag="T2c")
            nc.gpsimd.partition_broadcast(T2c[:], T2[:], channels=128)

            # ---------------- MSE ----------------
            for s in range(NSLAB):
                sl = slice(512 * s, 512 * (s + 1))
                prd = prd_pool.tile([128, W], F32, tag="prd")
                nc.sync.dma_start(
                    out=prd[:],
                    in_=prd_d[512 * b + 128 * s:512 * b + 128 * (s + 1), :])
                ge1 = scr_pool.tile([128, W], F32, tag="t0")
                nc.gpsimd.tensor_scalar(ge1[:], img[:, sl], T1c[:], None, OP.is_ge)
                ge2 = scr_pool.tile([128, W], F32, tag="t1")
                nc.vector.tensor_scalar(ge2[:], img[:, sl], T2c[:], None, OP.is_ge)
                nc.vector.scalar_tensor_tensor(ge2[:], ge1[:], 0.5, ge2[:], OP.mult, OP.max)
                pm = scr_pool.tile([128, W], F32, tag="t2")
                nc.vector.tensor_tensor(pm[:], prd[:], M[:, sl], OP.mult)
                nc.vector.tensor_tensor(pm[:], ge2[:], pm[:], OP.subtract)
                dsq = scr_pool.tile([128, W], F32, tag="t3")
                nc.vector.tensor_tensor_reduce(
                    dsq[:], pm[:], pm[:], 1.0, 0.0, OP.mult, OP.add,
                    accum_out=sq_cols[:, 4 * b + s:4 * b + s + 1])

        # ---------------- ship stats ----------------
        allc = stat_pool.tile([128, 32], F32, tag="allc")
        nc.vector.tensor_copy(out=allc[:, 0:16], in_=sq_cols[:])
        nc.vector.tensor_copy(out=allc[:, 16:32], in_=sm_cols[:])
        red = stat_pool.tile([1, 32], F32, tag="red")
        nc.gpsimd.tensor_reduce(red[:], allc[:], AX.C, OP.add)
        nc.sync.dma_start(out=out_d[:], in_=red[:])
        nc.sync.dma_start(out=dbg_d[:], in_=dbg_row[:])


_NC_CACHE = None


def _get_nc():
    global _NC_CACHE
    if _NC_CACHE is None:
        _NC_CACHE = build_nc()
    return _NC_CACHE


def kernel(preds, labels, images):
    preds = np.asarray(preds)
    labels = np.asarray(labels)
    images = np.asarray(images)
    B = preds.shape[0]
    assert B == 32 and preds.shape == (32, 1, 512, 512)
    nc = _get_nc()

    in_maps = []
    for c in range(8):
        sl = slice(B_PER_CORE * c, B_PER_CORE * (c + 1))
        in_maps.append({
            "labels": labels[sl, 0].reshape(B_PER_CORE * H, W),
            "images": images[sl, 0].reshape(B_PER_CORE * H, W),
            "preds": preds[sl, 0].reshape(B_PER_CORE * H, W),
        })
    res = run_bass_kernel_spmd(nc, in_maps, list(range(8)))

    # host: finish per-sample loss + mean over valid (np.float32 math)
    sq = np.zeros(32, np.float32)
    sm = np.zeros(32, np.float32)
    for c in range(8):
        st = res.results[c]["stats"][0]
        for b in range(B_PER_CORE):
            sq[B_PER_CORE * c + b] = np.sum(st[4 * b:4 * b + 4], dtype=np.float32)
            sm[B_PER_CORE * c + b] = np.sum(st[16 + 4 * b:16 + 4 * b + 4], dtype=np.float32)
    smp = sm + np.float32(EPS)
    valid = smp > np.float32(1e-8)
    loss_per = sq / smp
    cnt = np.float32(valid.sum())
    if cnt > 0:
        total = np.sum(np.where(valid, loss_per, np.float32(0.0)), dtype=np.float32)
        out = total / np.maximum(cnt, np.float32(1.0))
    else:
        out = np.float32(0.0)
    return np.float32(out)
